# revision 16
# baseline (speedup 1.0000x reference)
"""GatedGCN (4-layer, H=128) distributed Bass kernel for 8 TRN2 NeuronCores.

v2 architecture (race-free, no scatter-add):
  - Nodes sharded 8 x 12500 (padded to 12544 = 98*128 rows per core).
  - Edges partitioned by dst owner core and bucketed by aligned 128-node
    dst windows.  Each 128-token sub-tile is statically bound to ONE window
    (uniform quota across cores keeps the SPMD graph identical; pad tokens
    carry dstm=-1 and vanish through the is_equal masks).
  - B||D rows fetched per tile with one indirect_dma_start (int32 absolute
    rows, unique output slots).
  - Eh[dst] expand and num||den segment-sum are selection-mask matmuls
    against PSUM window tiles; flushes write a single SBUF num||den buffer.
  - Per layer: fused node pass (h update + next-layer GEMMs) -> AllGather
    of B||D [12544,256] bf16 -> edge pass.
"""
import os
import sys
import numpy as np

sys.path.insert(0, "/opt/trn_rl_repo")

HIDDEN = 128
N_LAYERS = 4
IN_NODE = 6
IN_EDGE = 2
ACT_DIM = 2
N_NODES = 100000
N_EDGES = 600000
N_CORES = 8
EPS = 1e-6
MAX_ACTION = 1.0

_last_exec_time_ns = None


def _full_cfg():
    return dict(npc=12500, npad=12544, tile=2048, n_layers=N_LAYERS)


# ----------------------------------------------------------------------------
# Host preprocessing
# ----------------------------------------------------------------------------

def preprocess(src, dst, cfg):
    """Bucket edges by (owner core, dst window); uniform per-window quotas.

    Returns (cores, sched, e_pad, n_tiles) where sched holds the uniform
    sub-tile -> window binding and window lifecycle.
    """
    npc, npad, tile = cfg["npc"], cfg["npad"], cfg["tile"]
    nnt = npad // 128                       # windows per core
    owner = dst // npc

    # per-core, per-window edge lists
    per_cw = []
    for c in range(N_CORES):
        eids = np.nonzero(owner == c)[0]
        dloc = dst[eids] % npc
        order = np.argsort(dloc, kind="stable")
        eids = eids[order]
        dloc = dloc[order]
        w = dloc // 128
        bounds = np.searchsorted(w, np.arange(nnt + 1))
        per_cw.append((eids, dloc, bounds))

    # uniform quotas (sub-tiles per window)
    q = np.zeros(nnt, np.int64)
    for k in range(nnt):
        m = max(pc[2][k + 1] - pc[2][k] for pc in per_cw)
        q[k] = max((m + 127) // 128, 1)
    nsub = int(q.sum())
    spt = tile // 128                       # sub-tiles per DMA tile
    # pad nsub to a tile multiple by growing the last window's quota
    extra = (-nsub) % spt
    q[nnt - 1] += extra
    nsub += extra
    n_tiles = nsub // spt
    e_pad = nsub * 128
    s_k = np.concatenate([[0], np.cumsum(q)])   # first sub-tile per window
    kofs = np.empty(nsub, np.int64)             # sub-tile -> window
    for k in range(nnt):
        kofs[s_k[k]:s_k[k + 1]] = k

    cores = []
    for c in range(N_CORES):
        eids, dloc, bounds = per_cw[c]
        perm = np.full(e_pad, -1, np.int64)
        dstm = np.full(e_pad, -1.0, np.float32)
        trow = np.zeros(e_pad, np.int64)
        for k in range(nnt):
            lo, hi = bounds[k], bounds[k + 1]
            t0 = s_k[k] * 128
            n = hi - lo
            perm[t0:t0 + n] = eids[lo:hi]
            dstm[t0:t0 + n] = (dloc[lo:hi] - 128 * k).astype(np.float32)
            s = src[eids[lo:hi]]
            trow[t0:t0 + n] = (s // npc) * npad + (s % npc)
        erow = np.full(e_pad, npad - 1, np.int64)
        for k in range(nnt):
            lo, hi = bounds[k], bounds[k + 1]
            t0 = s_k[k] * 128
            erow[t0:t0 + (hi - lo)] = dloc[lo:hi]
        cores.append(dict(perm=perm, dstm=dstm,
                          trow=trow.astype(np.int32),
                          erow=erow.astype(np.int32)))

    sched = dict(q=q, s_k=s_k, kofs=kofs, nsub=nsub, nnt=nnt)
    return cores, sched, e_pad, n_tiles


# ----------------------------------------------------------------------------
# Bass graph builder (one SPMD graph for all 8 cores)
# ----------------------------------------------------------------------------

def build(cfg, sched, e_pad, n_tiles, debug=False):
    import concourse.bass as bass
    import concourse.bacc as bacc
    import concourse.mybir as mybir
    import concourse.tile as tile_mod

    F32 = mybir.dt.float32
    BF16 = mybir.dt.bfloat16
    I32 = mybir.dt.int32
    AF = mybir.ActivationFunctionType
    AL = mybir.AluOpType

    npad, TILE, L = cfg["npad"], cfg["tile"], cfg["n_layers"]
    NT = n_tiles
    C = TILE // 128          # sub-tiles per edge tile (16)
    GE = 4                   # sub-tiles per ACT/PSUM group
    NNT = npad // 128        # node tiles / windows
    NSUB = sched["nsub"]
    KOFS = sched["kofs"]
    S_K = sched["s_k"]
    TAB_ROWS = npad * N_CORES

    nc = bacc.Bacc(None, target_bir_lowering=False, debug=debug)

    # ---- I/O ----
    nfT = nc.dram_tensor("nfT", [IN_NODE, npad], F32, kind="ExternalInput")
    efT = nc.dram_tensor("efT", [IN_EDGE, e_pad], BF16,
                         kind="ExternalInput")
    idx_d = nc.dram_tensor("idx32", [128, NSUB], I32, kind="ExternalInput")
    eidx_d = nc.dram_tensor("eidx32", [128, NSUB], I32,
                            kind="ExternalInput")
    dstm_d = nc.dram_tensor("dstm", [e_pad, 1], F32, kind="ExternalInput")
    wts = nc.dram_tensor("wts", [L, 5, 128, 128], BF16, kind="ExternalInput")
    embw = nc.dram_tensor("embw", [IN_NODE + IN_EDGE, 128], F32,
                          kind="ExternalInput")
    biases = nc.dram_tensor("biases", [3 * L + 3, 128, 128], F32,
                            kind="ExternalInput")
    consts = nc.dram_tensor("consts", [4, 128, 128], F32,
                            kind="ExternalInput")  # fiota_rep, eye, ones, piota
    outw_d = nc.dram_tensor("outw", [128, ACT_DIM], BF16, kind="ExternalInput")
    embe_d = nc.dram_tensor("embe_bf", [IN_EDGE, 128], BF16,
                            kind="ExternalInput")
    identb_d = nc.dram_tensor("identb", [128, 128], BF16, kind="ExternalInput")
    out_d = nc.dram_tensor("out", [npad, ACT_DIM], F32, kind="ExternalOutput")

    # ---- internal DRAM ----
    h_d = [nc.dram_tensor(f"h{i}", [npad, 128], F32) for i in range(2)]
    ah_d = [nc.dram_tensor(f"ah{i}", [npad, 128], BF16) for i in range(2)]
    eh_d = [nc.dram_tensor(f"eh{i}", [npad, 128], BF16) for i in range(2)]
    bnc_d = [nc.dram_tensor(f"bnc{i}", [npad, 256], BF16) for i in range(2)]
    tab_d = [nc.dram_tensor(f"tab{i}", [TAB_ROWS, 256], BF16,
                            addr_space="Shared") for i in range(2)]
    e_d = [nc.dram_tensor(f"e{i}", [e_pad, 128], BF16) for i in range(2)]

    def tm(t):
        return t.ap().rearrange("(n p) e -> p n e", p=128)

    h_v = [tm(t) for t in h_d]
    ah_v = [tm(t) for t in ah_d]
    eh_v = [tm(t) for t in eh_d]
    bnc_v = [tm(t) for t in bnc_d]
    e_v = [t.ap().rearrange("(n c p) e -> n p c e", p=128, c=C) for t in e_d]
    dstm_tm = dstm_d.ap().rearrange("(n p) o -> p n o", p=128)  # [128,NSUB,1]
    dstm_row = dstm_d.ap().rearrange("(n x) o -> n o x", x=TILE)  # [NT,1,TILE]

    rg = [list(range(N_CORES))]

    with tile_mod.TileContext(nc) as tc:
        with tc.tile_pool(name="const", bufs=1) as cp:
            wsb = cp.tile([128, L, 5, 128], BF16)
            for l in range(L):
                for k in range(5):
                    nc.sync.dma_start(wsb[:, l, k, :], wts[l, k])
            bsb = cp.tile([128, 3 * L + 3, 128], F32)
            for b in range(3 * L + 3):
                nc.sync.dma_start(bsb[:, b, :], biases[b])
            embs = cp.tile([IN_NODE, 128], F32)
            nc.sync.dma_start(embs[:], embw[0:IN_NODE, :])
            embe = cp.tile([IN_EDGE, 128], BF16)
            nc.sync.dma_start(embe[:], embe_d[:])
            identb = cp.tile([128, 128], BF16)
            nc.sync.dma_start(identb[:], identb_d[:])
            identf = cp.tile([128, 128], F32)
            nc.sync.dma_start(identf[:], consts[1])
            fiota = cp.tile([128, 1, 128], F32)
            nc.sync.dma_start(fiota[:, 0, :], consts[0])
            outw = cp.tile([128, ACT_DIM], BF16)
            nc.sync.dma_start(outw[:], outw_d[:])
            ndsb = cp.tile([128, NNT, 256], BF16)   # num||den windows
            epst = cp.tile([128, 1], F32)
            nc.vector.memset(epst[:], EPS)
            idxsb = cp.tile([128, NSUB], I32)
            nc.sync.dma_start(idxsb[:], idx_d[:])
            eidxsb = cp.tile([128, NSUB], I32)
            nc.sync.dma_start(eidxsb[:], eidx_d[:])
            dstmsb = cp.tile([128, NSUB, 1], F32)
            nc.sync.dma_start(dstmsb[:], dstm_tm)

            def W(l, k):
                return wsb[:, l, k, :]

            def BIAS(l, k):  # 0=A,1=B,2=e
                return bsb[:, 3 * l + k, :]

            B_EMBH = bsb[:, 3 * L, :]
            B_EMBE = bsb[:, 3 * L + 1, :]
            B_OUT = bsb[:, 3 * L + 2, 0:ACT_DIM]

            # ----------------------------------------------------------------
            def node_pass(l):
                lw = l + 1
                last = (l == L - 1)
                with (
                    tc.tile_pool(name=f"np{l}", bufs=3) as pp,
                    tc.tile_pool(name=f"npp{l}", bufs=1, space="PSUM") as qq,
                ):
                    for i0 in range(0, NNT, 4):
                        g = min(4, NNT - i0)
                        h_sb = pp.tile([128, g, 128], F32, tag="h_sb")
                        if l < 0:
                            nf_sb = pp.tile([IN_NODE, g * 128], F32, tag="nf")
                            nc.sync.dma_start(
                                nf_sb[:], nfT[:, i0 * 128:(i0 + g) * 128])
                            ph = qq.tile([128, g, 128], F32, tag="ph")
                            for t in range(g):
                                nc.tensor.matmul(
                                    ph[:, t, :],
                                    nf_sb[:, t * 128:(t + 1) * 128],
                                    embs[:], start=(t % 4 == 0),
                                    stop=(t % 4 == 3 or t == g - 1),
                                    skip_group_check=True)
                            for t in range(g):
                                nc.vector.tensor_add(
                                    h_sb[:, t, :], ph[:, t, :], B_EMBH)
                        else:
                            hp_sb = pp.tile([128, g, 128], F32, tag="hp")
                            nc.sync.dma_start(
                                hp_sb[:], h_v[l % 2][:, i0:i0 + g, :])
                            a_sb = pp.tile([128, g, 128], BF16, tag="a_sb")
                            nc.sync.dma_start(
                                a_sb[:], ah_v[l % 2][:, i0:i0 + g, :])
                            for t in range(g):
                                i = i0 + t
                                nd = ndsb[:, i, :]
                                de = pp.tile([128, 128], F32, tag="de")
                                nc.scalar.activation(
                                    de[:], nd[:, 128:256], AF.Identity,
                                    bias=epst[:])
                                rec = pp.tile([128, 128], F32, tag="rec")
                                nc.vector.reciprocal(rec[:], de[:])
                                tt = pp.tile([128, 128], F32, tag="tt")
                                nc.vector.tensor_mul(tt[:], nd[:, 0:128],
                                                     rec[:])
                                t2 = pp.tile([128, 128], F32, tag="t2")
                                nc.vector.tensor_add(
                                    t2[:], tt[:], a_sb[:, t, :])
                                rl = pp.tile([128, 128], F32, tag="rl")
                                nc.scalar.activation(rl[:], t2[:], AF.Relu)
                                nc.vector.tensor_add(
                                    h_sb[:, t, :], hp_sb[:, t, :], rl[:])
                        if not last:
                            nc.sync.dma_start(
                                h_v[(l + 1) % 2][:, i0:i0 + g, :], h_sb[:])
                        phT = qq.tile([128, g, 128], F32, tag="phT")
                        for t in range(g):
                            nc.tensor.matmul(
                                phT[:, t, :], h_sb[:, t, :], identf[:],
                                is_transpose=True, start=(t % 4 == 0),
                                stop=(t % 4 == 3 or t == g - 1),
                                skip_group_check=True)
                        hT = pp.tile([128, g, 128], BF16, tag="hT")
                        nc.scalar.copy(hT[:], phT[:])
                        if not last:
                            pbd = qq.tile([128, g, 256], F32, tag="pbd")
                            for t in range(g):
                                first = (t % 2 == 0)
                                lastb = (t % 2 == 1 or t == g - 1)
                                nc.tensor.matmul(pbd[:, t, 0:128], hT[:, t, :],
                                                 W(lw, 1), start=first,
                                                 stop=False,
                                                 skip_group_check=True)
                                nc.tensor.matmul(pbd[:, t, 128:256],
                                                 hT[:, t, :], W(lw, 2),
                                                 start=False, stop=lastb,
                                                 skip_group_check=True)
                            bd_sb = pp.tile([128, g, 256], BF16, tag="bd_sb")
                            for t in range(g):
                                nc.vector.tensor_add(
                                    bd_sb[:, t, 0:128], pbd[:, t, 0:128],
                                    BIAS(lw, 1))
                                nc.vector.tensor_copy(
                                    bd_sb[:, t, 128:256], pbd[:, t, 128:256])
                            nc.sync.dma_start(
                                bnc_v[lw % 2][:, i0:i0 + g, :], bd_sb[:])
                            pae = qq.tile([128, g, 256], F32, tag="pae")
                            for t in range(g):
                                first = (t % 2 == 0)
                                lastb = (t % 2 == 1 or t == g - 1)
                                nc.tensor.matmul(pae[:, t, 0:128], hT[:, t, :],
                                                 W(lw, 3), start=first,
                                                 stop=False,
                                                 skip_group_check=True)
                                nc.tensor.matmul(pae[:, t, 128:256],
                                                 hT[:, t, :], W(lw, 0),
                                                 start=False, stop=lastb,
                                                 skip_group_check=True)
                            ae_sb = pp.tile([128, g, 256], BF16, tag="ae_sb")
                            for t in range(g):
                                nc.vector.tensor_add(
                                    ae_sb[:, t, 0:128], pae[:, t, 0:128],
                                    BIAS(lw, 2))
                                nc.vector.tensor_add(
                                    ae_sb[:, t, 128:256], pae[:, t, 128:256],
                                    BIAS(lw, 0))
                            nc.sync.dma_start(
                                eh_v[lw % 2][:, i0:i0 + g, :],
                                ae_sb[:, :, 0:128])
                            nc.sync.dma_start(
                                ah_v[lw % 2][:, i0:i0 + g, :],
                                ae_sb[:, :, 128:256])
                        else:
                            po = qq.tile([128, g, ACT_DIM], F32, tag="pbd")
                            for t in range(g):
                                nc.tensor.matmul(po[:, t, :], hT[:, t, :],
                                                 outw[:], start=(t == 0),
                                                 stop=(t == g - 1),
                                                 skip_group_check=True)
                            o_sb = pp.tile([128, g, ACT_DIM], F32, tag="o_sb")
                            for t in range(g):
                                nc.vector.tensor_add(
                                    o_sb[:, t, :], po[:, t, :], B_OUT)
                            nc.vector.tensor_scalar(
                                o_sb[:], o_sb[:], MAX_ACTION, -MAX_ACTION,
                                AL.min, AL.max)
                            nc.sync.dma_start(
                                out_d.ap().rearrange(
                                    "(n p) e -> p n e", p=128)[:, i0:i0 + g, :],
                                o_sb[:])

            # ----------------------------------------------------------------
            def edge_pass(l):
                tab = tab_d[l % 2]
                ehd = eh_d[l % 2]
                ndw = {}
                with (
                    tc.tile_pool(name=f"ep{l}", bufs=3) as pp,
                    tc.tile_pool(name=f"em{l}", bufs=2) as mp,
                    tc.tile_pool(name=f"epp{l}", bufs=2, space="PSUM") as qq,
                    tc.tile_pool(name=f"epw{l}", bufs=2, space="PSUM") as qw,
                ):
                    for t in range(NT):
                        j0 = t * C
                        bd_g = pp.tile([128, C, 256], BF16, tag="bd_g", bufs=2)
                        for j in range(C):
                            nc.gpsimd.indirect_dma_start(
                                out=bd_g[:, j, :], out_offset=None,
                                in_=tab[:, :],
                                in_offset=bass.IndirectOffsetOnAxis(
                                    ap=idxsb[:, j0 + j:j0 + j + 1], axis=0))
                        e_t = pp.tile([128, C, 128], BF16, tag="e_t")
                        if l == 0:
                            ef_sb = pp.tile([IN_EDGE, TILE], BF16, tag="ef",
                                            bufs=2)
                            nc.sync.dma_start(
                                ef_sb[:], efT[:, t * TILE:(t + 1) * TILE])
                        else:
                            nc.sync.dma_start(e_t[:], e_v[l % 2][t])
                        # masks: one DVE op, maskT via PE transpose
                        mask = mp.tile([128, C, 128], BF16, tag="mask")
                        nc.vector.tensor_tensor(
                            out=mask[:],
                            in0=dstmsb[:, j0:j0 + C, :].to_broadcast(
                                [128, C, 128]),
                            in1=fiota[:].to_broadcast([128, C, 128]),
                            op=AL.is_equal)
                        eh_g = pp.tile([128, C, 128], BF16, tag="eh_g",
                                       bufs=2)
                        for j in range(C):
                            nc.gpsimd.indirect_dma_start(
                                out=eh_g[:, j, :], out_offset=None,
                                in_=ehd[:, :],
                                in_offset=bass.IndirectOffsetOnAxis(
                                    ap=eidxsb[:, j0 + j:j0 + j + 1], axis=0))
                        msig = mp.tile([128, C, 256], BF16, tag="msig")
                        for g0 in range(0, C, GE):
                            sl = slice(g0, g0 + GE)
                            if l == 0:
                                pe0 = qq.tile([128, GE, 128], F32, tag="pTT")
                                for j in range(GE):
                                    nc.tensor.matmul(
                                        pe0[:, j, :],
                                        ef_sb[:, (g0 + j) * 128:
                                              (g0 + j + 1) * 128],
                                        embe[:],
                                        start=(j == 0), stop=(j == GE - 1),
                                        skip_group_check=True)
                                for j in range(GE):
                                    nc.vector.tensor_add(
                                        e_t[:, g0 + j, :], pe0[:, j, :],
                                        B_EMBE)
                            pT = qq.tile([128, GE, 128], BF16, tag="pT")
                            for j in range(GE):
                                nc.tensor.matmul(
                                    pT[:, j, :], e_t[:, g0 + j, :], identb[:],
                                    is_transpose=True, start=(j == 0),
                                    stop=(j == GE - 1),
                                    skip_group_check=True)
                            eT = pp.tile([128, GE, 128], BF16, tag="eT")
                            nc.scalar.copy(eT[:], pT[:])
                            pe = qq.tile([128, GE, 128], F32, tag="pe")
                            for j in range(GE):
                                nc.tensor.matmul(
                                    pe[:, j, :], eT[:, j, :], W(l, 4),
                                    start=(j == 0), stop=False,
                                    skip_group_check=True)
                            nc.tensor.matmul(
                                pe[:], identb[:], bd_g[:, sl, 128:256],
                                start=False, stop=False, skip_group_check=True)
                            nc.tensor.matmul(
                                pe[:], identb[:], eh_g[:, sl, :],
                                start=False, stop=True, skip_group_check=True)
                            nc.scalar.activation(
                                msig[:, sl, 128:256], pe[:], AF.Sigmoid)
                            nc.vector.tensor_mul(
                                msig[:, sl, 0:128], msig[:, sl, 128:256],
                                bd_g[:, sl, 0:128])
                            # e_out = e_in + relu(e_new), fused on DVE
                            if l < L - 1:
                                nc.vector.scalar_tensor_tensor(
                                    out=e_t[:, sl, :], in0=pe[:],
                                    scalar=0.0, in1=e_t[:, sl, :],
                                    op0=AL.max, op1=AL.add)
                            # aggregation matmuls into window psum
                            for j in range(GE):
                                js = j0 + g0 + j
                                k = int(KOFS[js])
                                if k not in ndw:
                                    ndw[k] = qw.tile([128, 256], F32,
                                                     tag="ndw",
                                                     name=f"ndw{l}_{k}")
                                nc.tensor.matmul(
                                    ndw[k][:], mask[:, g0 + j, :],
                                    msig[:, g0 + j, :],
                                    start=(js == int(S_K[k])),
                                    stop=(js == int(S_K[k + 1]) - 1),
                                    skip_group_check=True)
                                if js == int(S_K[k + 1]) - 1:
                                    nc.scalar.copy(ndsb[:, k, :], ndw[k][:])
                                    del ndw[k]
                        if l < L - 1:
                            nc.sync.dma_start(e_v[(l + 1) % 2][t], e_t[:])

            # ---------------- program ----------------
            node_pass(-1)
            nc.gpsimd.collective_compute(
                "AllGather", nc_alu_bypass(nc), replica_groups=rg,
                ins=[bnc_d[0].ap().opt()], outs=[tab_d[0].ap().opt()])
            for l in range(L):
                edge_pass(l)
                node_pass(l)
                if l < L - 1:
                    nc.gpsimd.collective_compute(
                        "AllGather", nc_alu_bypass(nc), replica_groups=rg,
                        ins=[bnc_d[(l + 1) % 2].ap().opt()],
                        outs=[tab_d[(l + 1) % 2].ap().opt()])

    nc.compile()
    return nc


def nc_alu_bypass(nc):
    import concourse.mybir as mybir
    return mybir.AluOpType.bypass


# ----------------------------------------------------------------------------
# Host-side input assembly
# ----------------------------------------------------------------------------

def make_in_maps(inputs, cfg, cores, e_pad):
    import ml_dtypes
    bf = ml_dtypes.bfloat16
    npc, npad, L = cfg["npc"], cfg["npad"], cfg["n_layers"]
    nsub = e_pad // 128

    nf = np.asarray(inputs["node_feats"], np.float32)
    ef = np.asarray(inputs["edge_feats"], np.float32)

    wts = np.stack([np.stack([np.asarray(inputs[nm + "_w"][l], np.float32)
                              for nm in ["A", "B", "D", "E", "C"]])
                    for l in range(L)]).astype(bf)
    embw = np.concatenate([np.asarray(inputs["emb_h_w"], np.float32),
                           np.asarray(inputs["emb_e_w"], np.float32)], 0)
    nb = 3 * L + 3
    biases = np.zeros((nb, 128, 128), np.float32)
    for l in range(L):
        biases[3 * l + 0, :, :] = np.asarray(inputs["A_b"][l])[None, :]
        biases[3 * l + 1, :, :] = np.asarray(inputs["B_b"][l])[None, :]
        be = (np.asarray(inputs["D_b"][l]) + np.asarray(inputs["E_b"][l])
              + np.asarray(inputs["C_b"][l]))
        biases[3 * l + 2, :, :] = be[None, :]
    biases[3 * L, :, :] = np.asarray(inputs["emb_h_b"])[None, :]
    biases[3 * L + 1, :, :] = np.asarray(inputs["emb_e_b"])[None, :]
    biases[3 * L + 2, :, 0:ACT_DIM] = np.asarray(inputs["out_b"])[None, :]

    consts = np.zeros((4, 128, 128), np.float32)
    consts[0] = np.arange(128, dtype=np.float32)[None, :]   # fiota_rep
    consts[1] = np.eye(128, dtype=np.float32)
    consts[2, 0, :] = 1.0                                    # ones row
    consts[3, :, 0] = np.arange(128, dtype=np.float32)       # piota col

    outw = np.asarray(inputs["out_w"], np.float32).astype(bf)
    identb = np.eye(128, dtype=np.float32).astype(bf)

    in_maps = []
    for c in range(N_CORES):
        cc = cores[c]
        nfT = np.zeros((IN_NODE, npad), np.float32)
        nfT[:, :npc] = nf[c * npc:(c + 1) * npc].T
        efT = np.zeros((IN_EDGE, e_pad), np.float32)
        v = cc["perm"] >= 0
        efT[:, v] = ef[cc["perm"][v]].T
        efT = efT.astype(bf)
        idx32 = cc["trow"].reshape(nsub, 128).T.copy().astype(np.int32)
        eidx32 = cc["erow"].reshape(nsub, 128).T.copy().astype(np.int32)
        dstm = cc["dstm"].reshape(e_pad, 1)
        in_maps.append({
            "nfT": nfT, "efT": efT, "idx32": idx32, "eidx32": eidx32,
            "dstm": dstm,
            "wts": wts, "embw": embw.astype(np.float32), "biases": biases,
            "consts": consts, "outw": outw, "identb": identb,
            "embe_bf": embw[IN_NODE:].astype(bf),
        })
    return in_maps


# ----------------------------------------------------------------------------
# Entry point
# ----------------------------------------------------------------------------

def kernel(**inputs):
    global _last_exec_time_ns
    from concourse.bass_utils import run_bass_kernel_spmd

    cfg = _full_cfg()
    src = np.asarray(inputs["src"]).astype(np.int64)
    dst = np.asarray(inputs["dst"]).astype(np.int64)
    cores, sched, e_pad, n_tiles = preprocess(src, dst, cfg)
    nc = build(cfg, sched, e_pad, n_tiles)
    in_maps = make_in_maps(inputs, cfg, cores, e_pad)

    res = run_bass_kernel_spmd(nc, in_maps, list(range(N_CORES)))
    _last_exec_time_ns = res.exec_time_ns

    npc, npad = cfg["npc"], cfg["npad"]
    out = np.empty((N_NODES, ACT_DIM), np.float32)
    for c in range(N_CORES):
        out[c * npc:(c + 1) * npc] = res.results[c]["out"][:npc]
    return out
